# revision 15
# baseline (speedup 1.0000x reference)
"""MultiHeadGraphAttention Trainium2 Bass kernel.

Contract: kernel(**inputs) takes FULL unsharded inputs (np arrays, keyed as in
setup_inputs) and returns the FULL output tuple (out, attn), matching
reference() bit-for-bit in structure.

Sharding: query rows (N=512) are split 8 ways -> 64 rows/core/batch. Each core
gets its 64-row slice of query/mask/edge_features (for both batches), plus the
full key/value/weights, and produces its 64-row slice of `out` and `attn`.
No collectives; the host concatenates the row slices.
"""

import os

os.environ.setdefault("BASS_NEVER_TRACE", "1")

import numpy as np

import concourse.bacc as bacc
import concourse.bass as bass
import concourse.tile as tile
from concourse import mybir
from concourse.bass_utils import run_bass_kernel_spmd
from concourse.masks import make_identity

F32 = mybir.dt.float32
I32 = mybir.dt.int32
AF = mybir.ActivationFunctionType
ALU = mybir.AluOpType
AX = mybir.AxisListType

B = 2            # batch
N = 512          # sequence length
D = 512          # d_model
H = 8            # heads
DK = 64          # head dim
NCORES = 8
R = N // NCORES  # query rows per core per batch = 64
RT = B * R       # total rows per core = 128
P = 128          # partitions
TD = D // P      # 4 column-tiles of 128 in D (or N)
EROWS = 2        # edge rows loaded+reduced per step
LN_EPS = 1e-5
SCALE = 1.0 / np.sqrt(DK)


def _dup_rows_ap(ap2d, copies):
    """(p, f) AP -> (copies, p, f) AP broadcast along a new leading dim."""
    return bass.AP(tensor=ap2d.tensor, offset=ap2d.offset, ap=[[0, copies]] + list(ap2d.ap))


def _emit(tc, io):
    nc = tc.nc
    from contextlib import ExitStack

    with ExitStack() as ctx:
        const = ctx.enter_context(tc.tile_pool(name="const", bufs=1))
        persist = ctx.enter_context(tc.tile_pool(name="persist", bufs=1))
        edge_pool = ctx.enter_context(tc.tile_pool(name="edge", bufs=2))
        nat_pool = ctx.enter_context(tc.tile_pool(name="nat", bufs=3))
        xT_pool = ctx.enter_context(tc.tile_pool(name="xT", bufs=1))
        work = ctx.enter_context(tc.tile_pool(name="work", bufs=2))
        small = ctx.enter_context(tc.tile_pool(name="small", bufs=2))
        ps_mm = ctx.enter_context(tc.tile_pool(name="ps_mm", bufs=3, space="PSUM"))
        ps_tr = ctx.enter_context(tc.tile_pool(name="ps_tr", bufs=2, space="PSUM"))
        ps_cx = ctx.enter_context(tc.tile_pool(name="ps_cx", bufs=2, space="PSUM"))

        # ---------------- edge_features mean stream (the critical path) -------
        # biasT[b] accumulates column-sums: (128 m_local, 4 m_tile, 64 n)
        biasT = [
            persist.tile([P, TD, R], F32, tag=f"biasT{b}", name=f"biasT{b}")
            for b in range(B)
        ]
        for b in range(B):
            for n0 in range(0, R, EROWS):
                et = edge_pool.tile([P, EROWS, TD, D], F32, tag="edge")
                src = io["edge"][b, n0 : n0 + EROWS].rearrange(
                    "n (t p) d -> p n t d", p=P
                )
                nc.sync.dma_start(out=et[:], in_=src)
                outap = biasT[b][:, :, n0 : n0 + EROWS].rearrange("p t n -> p n t")
                nc.vector.reduce_sum(out=outap, in_=et[:], axis=AX.X)

        # ---------------- constants ----------------
        ident = const.tile([P, P], F32)
        ident_dram = nc.inline_tensor(np.eye(P, dtype=np.float32), name="ident_c")
        nc.sync.dma_start(out=ident[:], in_=ident_dram.ap())
        ones_row = const.tile([1, P], F32)
        nc.vector.memset(ones_row, 1.0)
        eps_col = const.tile([P, 1], F32)
        nc.vector.memset(eps_col, LN_EPS)

        bq_sb = const.tile([P, TD], F32)
        nc.sync.dma_start(out=bq_sb[:], in_=io["bq"].rearrange("(t p) -> p t", p=P))
        nc.vector.tensor_scalar_mul(bq_sb[:], bq_sb[:], float(SCALE))
        bk_sb = const.tile([P, TD], F32)
        nc.sync.dma_start(out=bk_sb[:], in_=io["bk"].rearrange("(t p) -> p t", p=P))
        bv_row = const.tile([1, D], F32)
        nc.sync.dma_start(out=bv_row[:], in_=io["bv"].rearrange("(o d) -> o d", o=1))
        bo_row = const.tile([1, D], F32)
        nc.sync.dma_start(out=bo_row[:], in_=io["bo"].rearrange("(o d) -> o d", o=1))
        lnw_bc = const.tile([P, D], F32)
        nc.sync.dma_start(out=lnw_bc[:], in_=_dup_rows_ap(io["ln_w"], P))
        lnb_bc = const.tile([P, D], F32)
        nc.sync.dma_start(out=lnb_bc[:], in_=_dup_rows_ap(io["ln_b"], P))

        # ---------------- weight transposes: W (dout,din) -> WT (din,dout) ----
        WT = {}
        for wname in ("Wq", "Wk", "Wv", "Wo"):
            wt = persist.tile([P, TD, D], F32, tag=f"{wname}T")
            WT[wname] = wt
            for dt in range(TD):  # dout tile
                wnat = nat_pool.tile([P, D], F32, tag="wnat")
                nc.sync.dma_start(out=wnat[:], in_=io[wname][dt * P : (dt + 1) * P, :])
                for di in range(TD):  # din tile
                    pst = ps_tr.tile([P, P], F32, tag="tr")
                    nc.tensor.transpose(pst[:], wnat[:, di * P : (di + 1) * P], ident[:])
                    nc.scalar.copy(out=wt[:, di, dt * P : (dt + 1) * P], in_=pst[:])

        # ---------------- query rows: natural + transposed -------------------
        q_nat = persist.tile([P, D], F32, tag="q_nat")  # 128 rows (b-major), d
        nc.sync.dma_start(out=q_nat[:], in_=io["query"].rearrange("b r d -> (b r) d"))
        queryT = persist.tile([P, TD, P], F32, tag="queryT")  # (din, din_t, row)
        for di in range(TD):
            pst = ps_tr.tile([P, P], F32, tag="tr")
            nc.tensor.transpose(pst[:], q_nat[:, di * P : (di + 1) * P], ident[:])
            nc.scalar.copy(out=queryT[:, di, :], in_=pst[:])

        # QT: (dout_local, dout_t, row) ; pre-scaled by 1/sqrt(dk), bias folded
        QT = persist.tile([P, TD, P], F32, tag="QT")
        for dt in range(TD):
            psq = ps_mm.tile([P, P], F32, tag="mm")
            for di in range(TD):
                nc.tensor.matmul(
                    psq[:],
                    WT["Wq"][:, di, dt * P : (dt + 1) * P],
                    queryT[:, di, :],
                    start=(di == 0),
                    stop=(di == TD - 1),
                )
            nc.scalar.activation(
                out=QT[:, dt, :], in_=psq[:], func=AF.Identity,
                bias=bq_sb[:, dt : dt + 1], scale=float(SCALE),
            )

        # ---------------- key/value transposes + K^T / V projections ---------
        KT = []  # per batch: (dout_local, dout_t, m)
        VN = []  # per batch: (m_local, m_t, dv)
        for b in range(B):
            keyT = xT_pool.tile([P, TD, N], F32, tag="keyT")
            valT = xT_pool.tile([P, TD, N], F32, tag="valT")
            for nt in range(TD):
                knat = nat_pool.tile([P, D], F32, tag="knat")
                nc.sync.dma_start(out=knat[:], in_=io["key"][b, nt * P : (nt + 1) * P, :])
                vnat = nat_pool.tile([P, D], F32, tag="vnat")
                nc.sync.dma_start(out=vnat[:], in_=io["value"][b, nt * P : (nt + 1) * P, :])
                for di in range(TD):
                    pst = ps_tr.tile([P, P], F32, tag="tr")
                    nc.tensor.transpose(pst[:], knat[:, di * P : (di + 1) * P], ident[:])
                    nc.scalar.copy(out=keyT[:, di, nt * P : (nt + 1) * P], in_=pst[:])
                    pst2 = ps_tr.tile([P, P], F32, tag="tr")
                    nc.tensor.transpose(pst2[:], vnat[:, di * P : (di + 1) * P], ident[:])
                    nc.scalar.copy(out=valT[:, di, nt * P : (nt + 1) * P], in_=pst2[:])

            if "dbg_keyT0" in io and b == 0:
                nc.sync.dma_start(out=io["dbg_keyT0"], in_=keyT[:])
            kt = persist.tile([P, TD, N], F32, tag=f"KT{b}")
            KT.append(kt)
            for dt in range(TD):
                psk = ps_mm.tile([P, N], F32, tag="mm")
                for di in range(TD):
                    nc.tensor.matmul(
                        psk[:],
                        WT["Wk"][:, di, dt * P : (dt + 1) * P],
                        keyT[:, di, :],
                        start=(di == 0),
                        stop=(di == TD - 1),
                    )
                nc.scalar.activation(
                    out=kt[:, dt, :], in_=psk[:], func=AF.Identity,
                    bias=bk_sb[:, dt : dt + 1], scale=1.0,
                )

            vn = persist.tile([P, TD, D], F32, tag=f"VN{b}")
            VN.append(vn)
            for mt in range(TD):
                psv = ps_mm.tile([P, N], F32, tag="mm")
                for di in range(TD):
                    nc.tensor.matmul(
                        psv[:],
                        valT[:, di, mt * P : (mt + 1) * P],
                        WT["Wv"][:, di, :],
                        start=(di == 0),
                        stop=False,
                    )
                # += broadcast(bv) as a rank-1 matmul
                nc.tensor.matmul(psv[:], ones_row[:], bv_row[:], start=False, stop=True)
                nc.scalar.copy(out=vn[:, mt, :], in_=psv[:])

        if "dbg_QT" in io:
            nc.sync.dma_start(out=io["dbg_qnat"], in_=q_nat[:])
            nc.sync.dma_start(out=io["dbg_ident"], in_=ident[:])
            nc.sync.dma_start(out=io["dbg_QT"], in_=QT[:])
            nc.sync.dma_start(out=io["dbg_KT0"], in_=KT[0][:])
            nc.sync.dma_start(out=io["dbg_VN0"], in_=VN[0][:])
            nc.sync.dma_start(out=io["dbg_queryT"], in_=queryT[:])
            nc.sync.dma_start(out=io["dbg_WqT"], in_=WT["Wq"][:])

        # ---------------- per-batch additive bias (edge mean + mask) ---------
        # bias_nm[b]: (128, N) fp32, rows 0:64 == rows 64:128 (head-pair dup)
        bias_nm = []
        for b in range(B):
            bnm = persist.tile([P, N], F32, tag=f"bias_nm{b}")
            bias_nm.append(bnm)
            for t in range(TD):
                pst = ps_tr.tile([P, P], F32, tag="tr")
                nc.tensor.transpose(pst[0:R, :], biasT[b][:, t, :], ident[:])
                nc.scalar.mul(bnm[0:R, t * P : (t + 1) * P], pst[0:R, 0:P], 1.0 / D)
                nc.scalar.mul(bnm[R:P, t * P : (t + 1) * P], pst[0:R, 0:P], 1.0 / D)
            mint = small.tile([P, N], I32, tag="mint")
            nc.sync.dma_start(out=mint[0:R, :], in_=io["mask"][b])
            nc.sync.dma_start(out=mint[R:P, :], in_=io["mask"][b])
            mflt = small.tile([P, N], F32, tag="mflt")
            nc.vector.tensor_copy(out=mflt[:], in_=mint[:])
            nc.vector.tensor_scalar(
                out=mflt[:], in0=mflt[:], scalar1=1e9, scalar2=-1e9,
                op0=ALU.mult, op1=ALU.add,
            )
            nc.vector.tensor_add(bnm[:], bnm[:], mflt[:])

        if "dbg_bias0" in io:
            nc.sync.dma_start(out=io["dbg_bias0"], in_=bias_nm[0][:])

        # ---------------- attention tail, per batch / head-pair --------------
        ctx_pair = persist.tile([P, D], F32, tag="ctx_pair")  # rows b-major, d
        for b in range(B):
            for t in range(TD):  # head pair (2t, 2t+1)
                pss = ps_mm.tile([P, N], F32, tag="mm")
                nc.tensor.matmul(
                    pss[0:R, :], QT[0:R, t, b * R : (b + 1) * R], KT[b][0:R, t, :],
                    start=True, stop=True, tile_position=(0, 0),
                )
                nc.tensor.matmul(
                    pss[R:P, :], QT[R:P, t, b * R : (b + 1) * R], KT[b][R:P, t, :],
                    start=True, stop=True, tile_position=(R, R),
                )
                sc = work.tile([P, N], F32, tag="sc")
                nc.vector.tensor_add(sc[:], pss[:], bias_nm[b][:])
                if "dbg_sc0" in io and b == 0 and t == 0:
                    nc.sync.dma_start(out=io["dbg_sc0"], in_=sc[:])
                mx = small.tile([P, 1], F32, tag="mx")
                nc.vector.reduce_max(out=mx[:], in_=sc[:], axis=AX.X)
                nmx = small.tile([P, 1], F32, tag="nmx")
                nc.vector.tensor_scalar_mul(nmx[:], mx[:], -1.0)
                ex = work.tile([P, N], F32, tag="ex")
                es = small.tile([P, 1], F32, tag="es")
                nc.scalar.activation(
                    out=ex[:], in_=sc[:], func=AF.Exp, bias=nmx[:, 0:1], scale=1.0,
                    accum_out=es[:, 0:1],
                )
                rs = small.tile([P, 1], F32, tag="rs")
                nc.vector.reciprocal(rs[:], es[:])
                at = work.tile([P, N], F32, tag="at")
                nc.vector.tensor_scalar_mul(at[:], ex[:], rs[:, 0:1])
                nc.sync.dma_start(out=io["attn_part"][b, 2 * t], in_=at[0:R, :])
                nc.sync.dma_start(out=io["attn_part"][b, 2 * t + 1], in_=at[R:P, :])

                # transpose attn -> (m, n-pair), then ctx for both heads
                psc = ps_cx.tile([P, DK], F32, tag="cx")
                for mt in range(TD):
                    pst = ps_tr.tile([P, P], F32, tag="tr")
                    nc.tensor.transpose(pst[:], at[:, mt * P : (mt + 1) * P], ident[:])
                    att = small.tile([P, P], F32, tag="att")
                    nc.scalar.copy(out=att[:], in_=pst[:])
                    nc.tensor.matmul(
                        psc[0:R, :], att[:, 0:R],
                        VN[b][:, mt, 2 * t * DK : (2 * t + 1) * DK],
                        start=(mt == 0), stop=(mt == TD - 1),
                        tile_position=(0, 0), skip_group_check=True,
                    )
                    nc.tensor.matmul(
                        psc[R:P, :], att[:, R:P],
                        VN[b][:, mt, (2 * t + 1) * DK : (2 * t + 2) * DK],
                        start=(mt == 0), stop=(mt == TD - 1),
                        tile_position=(0, R), skip_group_check=True,
                    )
                nc.scalar.copy(
                    out=ctx_pair[b * R : (b + 1) * R, 2 * t * DK : (2 * t + 1) * DK],
                    in_=psc[0:R, :],
                )
                nc.scalar.copy(
                    out=ctx_pair[b * R : (b + 1) * R, (2 * t + 1) * DK : (2 * t + 2) * DK],
                    in_=psc[R:P, :],
                )

        # ---------------- output projection + residual + LayerNorm -----------
        ctxT = persist.tile([P, TD, P], F32, tag="ctxT")
        for di in range(TD):
            pst = ps_tr.tile([P, P], F32, tag="tr")
            nc.tensor.transpose(pst[:], ctx_pair[:, di * P : (di + 1) * P], ident[:])
            nc.scalar.copy(out=ctxT[:, di, :], in_=pst[:])
        pso = ps_mm.tile([P, D], F32, tag="mm")
        for di in range(TD):
            nc.tensor.matmul(
                pso[:], ctxT[:, di, :], WT["Wo"][:, di, :],
                start=(di == 0), stop=False,
            )
        nc.tensor.matmul(pso[:], ones_row[:], bo_row[:], start=False, stop=True)

        x_sb = work.tile([P, D], F32, tag="x_sb")
        nc.vector.tensor_add(x_sb[:], pso[:], q_nat[:])
        stats = small.tile([P, 6], F32, tag="stats")
        nc.vector.bn_stats(out=stats[:], in_=x_sb[:])
        mv = small.tile([P, 2], F32, tag="mv")
        nc.vector.bn_aggr(out=mv[:], in_=stats[:])
        sdv = small.tile([P, 1], F32, tag="sdv")
        nc.scalar.activation(
            out=sdv[:], in_=mv[:, 1:2], func=AF.Sqrt, bias=eps_col[:, 0:1]
        )
        rstd = small.tile([P, 1], F32, tag="rstd")
        nc.vector.reciprocal(rstd[:], sdv[:])
        xn = work.tile([P, D], F32, tag="xn")
        nc.vector.tensor_scalar(
            out=xn[:], in0=x_sb[:], scalar1=mv[:, 0:1], scalar2=rstd[:, 0:1],
            op0=ALU.subtract, op1=ALU.mult,
        )
        nc.vector.tensor_mul(xn[:], xn[:], lnw_bc[:])
        nc.vector.tensor_add(xn[:], xn[:], lnb_bc[:])
        nc.sync.dma_start(out=io["out_part"].rearrange("b r d -> (b r) d"), in_=xn[:])


def build(iters=1):
    nc = bacc.Bacc("TRN2", target_bir_lowering=False, debug=False, num_devices=NCORES)
    io = {
        "query": nc.dram_tensor("query", (B, R, D), F32, kind="ExternalInput").ap(),
        "key": nc.dram_tensor("key", (B, N, D), F32, kind="ExternalInput").ap(),
        "value": nc.dram_tensor("value", (B, N, D), F32, kind="ExternalInput").ap(),
        "edge": nc.dram_tensor("edge", (B, R, N, D), F32, kind="ExternalInput").ap(),
        "mask": nc.dram_tensor("mask", (B, R, N), I32, kind="ExternalInput").ap(),
        "Wq": nc.dram_tensor("Wq", (D, D), F32, kind="ExternalInput").ap(),
        "Wk": nc.dram_tensor("Wk", (D, D), F32, kind="ExternalInput").ap(),
        "Wv": nc.dram_tensor("Wv", (D, D), F32, kind="ExternalInput").ap(),
        "Wo": nc.dram_tensor("Wo", (D, D), F32, kind="ExternalInput").ap(),
        "bq": nc.dram_tensor("bq", (D,), F32, kind="ExternalInput").ap(),
        "bk": nc.dram_tensor("bk", (D,), F32, kind="ExternalInput").ap(),
        "bv": nc.dram_tensor("bv", (D,), F32, kind="ExternalInput").ap(),
        "bo": nc.dram_tensor("bo", (D,), F32, kind="ExternalInput").ap(),
        "ln_w": nc.dram_tensor("ln_w", (D,), F32, kind="ExternalInput").ap(),
        "ln_b": nc.dram_tensor("ln_b", (D,), F32, kind="ExternalInput").ap(),
        "out_part": nc.dram_tensor("out_part", (B, R, D), F32, kind="ExternalOutput").ap(),
        "attn_part": nc.dram_tensor("attn_part", (B, H, R, N), F32, kind="ExternalOutput").ap(),
    }
    with tile.TileContext(nc) as tc:
        for _ in range(iters):
            _emit(tc, io)
    nc.compile()
    return nc


_CACHE = {}


def _get_nc(iters=1):
    if iters not in _CACHE:
        _CACHE[iters] = build(iters)
    return _CACHE[iters]


def make_in_maps(inputs):
    ins = {k: np.asarray(v) for k, v in inputs.items()}
    in_maps = []
    for c in range(NCORES):
        r0 = c * R
        in_maps.append({
            "query": np.ascontiguousarray(ins["query"][:, r0 : r0 + R, :], np.float32),
            "key": np.ascontiguousarray(ins["key"], np.float32),
            "value": np.ascontiguousarray(ins["value"], np.float32),
            "edge": np.ascontiguousarray(ins["edge_features"][:, r0 : r0 + R], np.float32),
            "mask": np.ascontiguousarray(ins["mask"][:, r0 : r0 + R, :], np.int32),
            "Wq": np.ascontiguousarray(ins["Wq"], np.float32),
            "Wk": np.ascontiguousarray(ins["Wk"], np.float32),
            "Wv": np.ascontiguousarray(ins["Wv"], np.float32),
            "Wo": np.ascontiguousarray(ins["Wo"], np.float32),
            "bq": np.ascontiguousarray(ins["bq"], np.float32),
            "bk": np.ascontiguousarray(ins["bk"], np.float32),
            "bv": np.ascontiguousarray(ins["bv"], np.float32),
            "bo": np.ascontiguousarray(ins["bo"], np.float32),
            "ln_w": np.ascontiguousarray(ins["ln_w"], np.float32),
            "ln_b": np.ascontiguousarray(ins["ln_b"], np.float32),
        })
    return in_maps


def gather(results):
    out = np.concatenate([r["out_part"] for r in results], axis=1)
    attn = np.concatenate([r["attn_part"] for r in results], axis=2)
    return out, attn


def kernel(**inputs):
    nc = _get_nc(1)
    in_maps = make_in_maps(inputs)
    res = run_bass_kernel_spmd(nc, in_maps, core_ids=list(range(NCORES)))
    return gather(res.results)
